# revision 1
# baseline (speedup 1.0000x reference)
"""Trainium2 Bass kernel for the batched 2D Kalman filter (nn_KalmanFilterWrapper).

Math
----
The reference runs, per trajectory, a Kalman filter over T=4096 steps with a
constant-velocity model.  The gain/covariance recursion (Riccati) is
data-independent, so the scan collapses to a linear time-varying recurrence

    x_t = A_t x_{t-1} + k_t z_t,        y_t = x_t[0]

with coefficients shared across the whole batch.  The 4-state filter decouples
into two identical 2-state (position, velocity) scalar filters — one per
coordinate — giving B*2 = 8192 independent scalar sequences.

Blocking time into chunks of C=126 steps turns the whole filter into one
[128x128] @ [128xN] matmul per block: the contraction covers the block's 126
measurements plus 2 "carry" rows holding the filter state from the previous
block; output rows are the block's 126 positions plus duplicated (p_last,
v_last) rows that become the next block's carry.  All coefficient matrices are
precomputed on the host in float64.

Partition layout (all compute-engine accesses start at partition 0/64):
  contract rows: 0..1 = carry (p_prev, v_prev), 2+j = z_j
  output rows:   0 = p_last (dup), 1 = v_last, 2+j = p_j
The last (short, 64-step) block reads the final 126 input rows with zero
coefficients on the first 62, so no memset/padding is needed.

Sharding: data-parallel across 8 NeuronCores, 512 trajectories (1024 scalar
sequences) per core.  Layout on device is [time, sequence]; the host
transposes in/out of the reference's [batch, time, 2] layout.
"""

import numpy as np

import concourse.bass as bass
import concourse.bacc as bacc
import concourse.mybir as mybir
from concourse.bass_utils import run_bass_kernel_spmd
from concourse.tile import TileContext

# Problem constants (hardcoded per harness contract).
B = 4096
T = 4096
DT = 1.0
PROCESS_VARIANCE = 1e-05
MEASUREMENT_VARIANCE = 0.1
INIT_ERROR = 1.0

N_CORES = 8
NCOLS = (B * 2) // N_CORES  # 1024 scalar sequences per core
MAIN_C = 126                # block size; contract dim = C + 2 = 128
CHUNK = 512                 # matmul moving free-dim (fp32 max, one PSUM bank)

DT_F32 = mybir.dt.float32
USE_F32R = False  # fp32r: full-rate PE matmul (vs 2-pass fp32), ~1e-4 rel err
DT_F32R = mybir.dt.float32r if USE_F32R else mybir.dt.float32


def _blocks():
    """Returns [(t0_dma, n_skip)]; each block reads z[t0_dma : t0_dma+126] and
    filters steps t0_dma+n_skip .. t0_dma+125 (n_skip leading rows get zero
    coefficients)."""
    out = []
    t0 = 0
    while t0 + MAIN_C <= T:
        out.append((t0, 0))
        t0 += MAIN_C
    if t0 < T:
        rem = T - t0
        out.append((T - MAIN_C, MAIN_C - rem))
    return out


def _precompute_lhsT():
    """Host-side Riccati + per-block coefficient matrices, float64 -> f32.

    Returns [128, n_blocks*128] f32; block bi's stationary operand (lhsT) is
    cols [bi*128, (bi+1)*128): lhsT[k_contract, m_out] = U[m, k].
    """
    F = np.array([[1.0, DT], [0.0, 1.0]], dtype=np.float64)
    I2 = np.eye(2, dtype=np.float64)
    P = INIT_ERROR * I2.copy()
    A = np.zeros((T, 2, 2), dtype=np.float64)
    k = np.zeros((T, 2), dtype=np.float64)
    for t in range(T):
        Pp = F @ P @ F.T + PROCESS_VARIANCE * I2
        s = Pp[0, 0] + MEASUREMENT_VARIANCE
        kt = Pp[:, 0] / s
        k[t] = kt
        KH = np.zeros((2, 2), dtype=np.float64)
        KH[:, 0] = kt
        P = (I2 - KH) @ Pp
        A[t] = (I2 - KH) @ F

    blocks = _blocks()
    lhsT_all = np.zeros((128, len(blocks) * 128), dtype=np.float64)
    for bi, (t0, n_skip) in enumerate(blocks):
        # contract col of z_j within this block's tile: block 0 loads z[0:128]
        # into partitions 0..127 (no carry), others load z into 2..127.
        zcol = (lambda j: j) if bi == 0 else (lambda j: 2 + j)
        Rc = np.zeros((2, 128), dtype=np.float64)
        if bi == 0:
            Rc[0, 0] = 1.0  # x_{-1} = [z_0, 0]
        else:
            Rc[0, 0] = 1.0  # carry row 0 = p_prev
            Rc[1, 1] = 1.0  # carry row 1 = v_prev
        U = np.zeros((128, 128), dtype=np.float64)
        for j in range(MAIN_C):
            if j >= n_skip:
                t = t0 + j
                Rc = A[t] @ Rc
                Rc[:, zcol(j)] += k[t]
            U[2 + j, :] = Rc[0, :]
        U[0, :] = Rc[0, :]  # p_last (dup) -> next block carry row 0
        U[1, :] = Rc[1, :]  # v_last      -> next block carry row 1
        lhsT_all[:, bi * 128:(bi + 1) * 128] = U.T
    return np.ascontiguousarray(lhsT_all.astype(np.float32))


def _build_nc():
    blocks = _blocks()
    nblk = len(blocks)
    nchunks = NCOLS // CHUNK

    nc = bacc.Bacc()
    z = nc.dram_tensor("z", [T, NCOLS], DT_F32R, kind="ExternalInput")
    u = nc.dram_tensor("u", [128, nblk * 128], DT_F32R, kind="ExternalInput")
    v = nc.dram_tensor("v", [T, NCOLS], DT_F32, kind="ExternalOutput")

    # Group blocks into DMA units: blocks 0 and 1 are singles (block 0 has the
    # special full-128-row load), then pairs, then the short last block single.
    # Paired units move 1 MB per dma_start instead of 516 KB.
    units = [[0], [1]]
    bi = 2
    while bi + 1 < nblk - 1:
        units.append([bi, bi + 1])
        bi += 2
    while bi < nblk:
        units.append([bi])
        bi += 1

    with TileContext(nc) as tc:
        with (
            tc.tile_pool(name="consts", bufs=1) as cpool,
            tc.tile_pool(name="zpool", bufs=4) as zpool,
            tc.tile_pool(name="vpool", bufs=3) as vpool,
            tc.tile_pool(name="psum", bufs=4, space="PSUM") as ppool,
        ):
            u_tile = cpool.tile([128, nblk * 128], DT_F32R)
            nc.sync.dma_start(u_tile[:, :], u[:, :])

            # z tiles per unit; ztile[bi] = (tile, sub-index)
            ztile = {}
            for unit in units:
                zp = zpool.tile([128, 2, NCOLS], DT_F32R, tag="zp")
                if len(unit) == 2:
                    t0 = blocks[unit[0]][0]
                    src = z[t0:t0 + 2 * MAIN_C, :].rearrange(
                        "(b r) c -> r b c", b=2
                    )
                    nc.sync.dma_start(zp[2:128, :, :], src)
                elif unit[0] == 0:
                    # block 0 has no carry: load z[0:128] into all partitions
                    # (rows 126..127 get zero coefficients) — no memset needed
                    nc.sync.dma_start(zp[:, 0, :], z[0:128, :])
                else:
                    t0 = blocks[unit[0]][0]
                    nc.sync.dma_start(zp[2:128, 0, :], z[t0:t0 + MAIN_C, :])
                for si, b in enumerate(unit):
                    ztile[b] = (zp, si)

            for unit in units:
                vout = vpool.tile([128, 2, NCOLS], DT_F32, tag="vout")
                for si, b in enumerate(unit):
                    zp, zsub = ztile[b]
                    for ci in range(nchunks):
                        cols = bass.ds(ci * CHUNK, CHUNK)
                        ps = ppool.tile([128, CHUNK], DT_F32)
                        nc.tensor.matmul(
                            ps[:, :],
                            u_tile[:, bass.ds(b * 128, 128)],
                            zp[:, zsub, cols],
                            start=True,
                            stop=True,
                        )
                        if b + 1 < nblk:
                            zn, nsub = ztile[b + 1]
                            nc.scalar.copy(zn[0:2, nsub, cols], ps[0:2, :])
                        # split evictions across ACT and DVE
                        if ci % 2 == 0:
                            nc.scalar.copy(vout[:, si, cols], ps[:, :])
                        else:
                            nc.vector.tensor_copy(vout[:, si, cols], ps[:, :])
                if len(unit) == 2:
                    t0 = blocks[unit[0]][0]
                    dst = v[t0:t0 + 2 * MAIN_C, :].rearrange(
                        "(b r) c -> r b c", b=2
                    )
                    nc.sync.dma_start(dst, vout[2:128, :, :])
                else:
                    t0, n_skip = blocks[unit[0]]
                    nc.sync.dma_start(
                        v[t0 + n_skip:t0 + MAIN_C, :],
                        vout[2 + n_skip:128, 0, :],
                    )
    nc.finalize()  # Bacc.compile(): splits multi-waits, allocates registers
    return nc


_CACHE = {}


def _run(x_seq: np.ndarray, trace: bool = False):
    if "nc" not in _CACHE:
        _CACHE["nc"] = _build_nc()
        _CACHE["u"] = _precompute_lhsT()
    nc = _CACHE["nc"]
    u_all = _CACHE["u"]

    x = np.ascontiguousarray(np.asarray(x_seq, dtype=np.float32))
    assert x.shape == (B, T, 2), x.shape

    # [B, T, 2] -> [T, B*2]; column n = 2*b + c
    zt = np.ascontiguousarray(x.transpose(1, 0, 2).reshape(T, B * 2))

    in_maps = [
        {"z": np.ascontiguousarray(zt[:, i * NCOLS:(i + 1) * NCOLS]), "u": u_all}
        for i in range(N_CORES)
    ]
    res = run_bass_kernel_spmd(nc, in_maps, core_ids=list(range(N_CORES)), trace=trace)

    vt = np.concatenate([r["v"] for r in res.results], axis=1)  # [T, B*2]
    out = np.ascontiguousarray(vt.reshape(T, B, 2).transpose(1, 0, 2))
    return out, res


def kernel(x_seq: np.ndarray) -> np.ndarray:
    out, _ = _run(x_seq, trace=False)
    return out



# revision 3
# speedup vs baseline: 2.0970x; 2.0970x over previous
"""Trainium2 Bass kernel for the batched 2D Kalman filter (nn_KalmanFilterWrapper).

Math
----
The reference runs, per trajectory, a Kalman filter over T=4096 steps with a
constant-velocity model.  The gain/covariance recursion (Riccati) is
data-independent, so the scan collapses to a linear time-varying recurrence

    x_t = A_t x_{t-1} + k_t z_t,        y_t = x_t[0]

with coefficients shared across the whole batch.  The 4-state filter decouples
into two identical 2-state (position, velocity) scalar filters — one per
coordinate — giving B*2 = 8192 independent scalar sequences.

The recurrence coefficients converge to steady state by t~135, and the steady
transition matrix has spectral radius 0.9315, so the filter's impulse response
g_d decays below 1e-6 by d=192.  Each aligned 128-step output chunk therefore
depends (to ~1e-5, vs a 2e-2 accuracy gate) only on the 256 measurements in
its own and the preceding 128-step input block:

    y[128*ci : 128*(ci+1)] = W_lo @ z_prev_block + W_hi @ z_this_block

where (W_lo, W_hi) are one shared Toeplitz pair built from g for all ci >= 2,
exact time-varying matrices for ci == 1, and a single exact lower-triangular
matrix for ci == 0 (which also folds in the x0 = [z_0, 0] initial condition).
All 32 chunks are INDEPENDENT matmuls — no serial carry chain at all.

Everything (measurements, weights, outputs) is bf16 on the wire; matmuls
accumulate in fp32 PSUM.  Host-side float64 weight construction + end-to-end
numpy simulation puts the total l2 relative error at ~3e-3 (truncation alone:
2.4e-5).

Sharding: data-parallel across 8 NeuronCores, 512 trajectories (1024 scalar
sequences) per core.  Layout on device is [time, sequence]; the host
transposes/casts in and out of the reference's [batch, time, 2] fp32 layout.
DMA is batched in 1 MiB transfers (4 x 128 time rows); input DMAs ride the
sync HWDGE ring, output DMAs the scalar ring so they never queue behind each
other.
"""

import numpy as np
import ml_dtypes

import concourse.bass as bass
import concourse.bacc as bacc
import concourse.mybir as mybir
from concourse.bass_utils import run_bass_kernel_spmd
from concourse.tile import TileContext

# Problem constants (hardcoded per harness contract).
B = 4096
T = 4096
DT = 1.0
PROCESS_VARIANCE = 1e-05
MEASUREMENT_VARIANCE = 0.1
INIT_ERROR = 1.0

N_CORES = 8
NCOLS = (B * 2) // N_CORES  # 1024 scalar sequences per core
CHUNK = 512                 # matmul free dim (one fp32 PSUM bank)
GROUP = 4                   # 128-row blocks per DMA transfer (1 MiB)
NBLK = T // 128             # 32 output chunks
NGRP = NBLK // GROUP        # 8 DMA groups
NSLOT = 5                   # weight matrices: W0, Wlo1, Whi1, WloS, WhiS

BF16 = mybir.dt.bfloat16
F32 = mybir.dt.float32
NPBF16 = ml_dtypes.bfloat16


def _precompute_lhsT():
    """Host-side Riccati + chunk weight matrices, float64 -> bf16.

    Returns [128, 5*128] bf16; slot s holds lhsT = W_s.T so that
    matmul(out, lhsT, z) computes out[t, n] = sum_k W_s[t, k] z[k, n].
    """
    F = np.array([[1.0, DT], [0.0, 1.0]], dtype=np.float64)
    I2 = np.eye(2, dtype=np.float64)
    P = INIT_ERROR * I2.copy()
    A = np.zeros((T, 2, 2), dtype=np.float64)
    k = np.zeros((T, 2), dtype=np.float64)
    for t in range(T):
        Pp = F @ P @ F.T + PROCESS_VARIANCE * I2
        s = Pp[0, 0] + MEASUREMENT_VARIANCE
        kt = Pp[:, 0] / s
        k[t] = kt
        KH = np.zeros((2, 2), dtype=np.float64)
        KH[:, 0] = kt
        P = (I2 - KH) @ Pp
        A[t] = (I2 - KH) @ F

    # Exact input->output operator over the first 256 steps.  Rc[:, j] is the
    # coefficient of measurement z_j in the current state; the initial state
    # is x_{-1} = [z_0, 0].
    W = np.zeros((256, 256), dtype=np.float64)
    Rc = np.zeros((2, 256), dtype=np.float64)
    Rc[0, 0] = 1.0
    for t in range(256):
        Rc = A[t] @ Rc
        Rc[:, t] += k[t]
        W[t] = Rc[0]

    # Steady-state impulse response g_d = [Ainf^d kinf][0].
    g = np.zeros(256, dtype=np.float64)
    vv = k[-1].copy()
    for d in range(256):
        g[d] = vv[0]
        vv = A[-1] @ vv
    m, kk = np.mgrid[0:128, 0:128]
    WloS = g[m + 128 - kk]
    WhiS = np.where(m >= kk, g[np.abs(m - kk)], 0.0)

    slots = [
        W[0:128, 0:128],      # chunk 0 (exact, incl. initial condition)
        W[128:256, 0:128],    # chunk 1 lo (exact transient)
        W[128:256, 128:256],  # chunk 1 hi
        WloS,                 # chunks 2..31 lo (steady Toeplitz)
        WhiS,                 # chunks 2..31 hi
    ]
    lhsT = np.zeros((128, NSLOT * 128), dtype=np.float64)
    for s, Ws in enumerate(slots):
        lhsT[:, s * 128:(s + 1) * 128] = Ws.T
    return np.ascontiguousarray(lhsT.astype(NPBF16))


def _build_nc():
    nc = bacc.Bacc()
    z = nc.dram_tensor("z", [T, NCOLS], BF16, kind="ExternalInput")
    u = nc.dram_tensor("u", [128, NSLOT * 128], BF16, kind="ExternalInput")
    v = nc.dram_tensor("v", [T, NCOLS], BF16, kind="ExternalOutput")

    nchunks = NCOLS // CHUNK

    with TileContext(nc) as tc:
        with (
            tc.tile_pool(name="consts", bufs=1) as cpool,
            tc.tile_pool(name="zpool", bufs=4) as zpool,
            tc.tile_pool(name="vpool", bufs=3) as vpool,
            tc.tile_pool(name="psum", bufs=8, space="PSUM") as ppool,
        ):
            u_tile = cpool.tile([128, NSLOT * 128], BF16)
            nc.sync.dma_start(u_tile[:, :], u[:, :])

            # 1 MiB input loads: [512 time rows, NCOLS] -> [128, 4, NCOLS].
            ztiles = []
            for gi in range(NGRP):
                zp = zpool.tile([128, GROUP, NCOLS], BF16, tag="zp")
                src = z[gi * GROUP * 128:(gi + 1) * GROUP * 128, :].rearrange(
                    "(b r) c -> r b c", b=GROUP
                )
                nc.sync.dma_start(zp[:, :, :], src)
                ztiles.append(zp)

            for gi in range(NGRP):
                vout = vpool.tile([128, GROUP, NCOLS], BF16, tag="vout")
                for sub in range(GROUP):
                    ci = gi * GROUP + sub
                    zhi = ztiles[gi]
                    ps = [
                        ppool.tile([128, CHUNK], F32, name=f"ps{cc}", tag="ps")
                        for cc in range(nchunks)
                    ]
                    if ci == 0:
                        for cc in range(nchunks):
                            cols = bass.ds(cc * CHUNK, CHUNK)
                            nc.tensor.matmul(
                                ps[cc][:, :],
                                u_tile[:, bass.ds(0, 128)],
                                zhi[:, sub, cols],
                                start=True,
                                stop=True,
                            )
                    else:
                        if sub > 0:
                            zlo, losub = ztiles[gi], sub - 1
                        else:
                            zlo, losub = ztiles[gi - 1], GROUP - 1
                        lo_slot, hi_slot = (1, 2) if ci == 1 else (3, 4)
                        # lo over both col-chunks, then hi: consecutive
                        # matmuls share the stationary operand.
                        for cc in range(nchunks):
                            cols = bass.ds(cc * CHUNK, CHUNK)
                            nc.tensor.matmul(
                                ps[cc][:, :],
                                u_tile[:, bass.ds(lo_slot * 128, 128)],
                                zlo[:, losub, cols],
                                start=True,
                                stop=False,
                            )
                        for cc in range(nchunks):
                            cols = bass.ds(cc * CHUNK, CHUNK)
                            nc.tensor.matmul(
                                ps[cc][:, :],
                                u_tile[:, bass.ds(hi_slot * 128, 128)],
                                zhi[:, sub, cols],
                                start=False,
                                stop=True,
                            )
                    # split PSUM evictions across ACT and DVE
                    for cc in range(nchunks):
                        cols = bass.ds(cc * CHUNK, CHUNK)
                        if cc % 2 == 0:
                            nc.scalar.copy(vout[:, sub, cols], ps[cc][:, :])
                        else:
                            nc.vector.tensor_copy(vout[:, sub, cols], ps[cc][:, :])
                # 1 MiB output store on the scalar HWDGE ring (input loads
                # ride the sync ring, so the two never queue behind each
                # other).
                dst = v[gi * GROUP * 128:(gi + 1) * GROUP * 128, :].rearrange(
                    "(b r) c -> r b c", b=GROUP
                )
                nc.scalar.dma_start(dst, vout[:, :, :])
    nc.finalize()  # Bacc.compile(): splits multi-waits, allocates registers
    return nc


_CACHE = {}


def _run(x_seq: np.ndarray, trace: bool = False):
    if "nc" not in _CACHE:
        _CACHE["nc"] = _build_nc()
        _CACHE["u"] = _precompute_lhsT()
    nc = _CACHE["nc"]
    u_all = _CACHE["u"]

    x = np.asarray(x_seq, dtype=np.float32)
    assert x.shape == (B, T, 2), x.shape

    # [B, T, 2] -> [T, B*2] bf16; column n = 2*b + c
    zt = np.ascontiguousarray(x.transpose(1, 0, 2).reshape(T, B * 2)).astype(NPBF16)

    in_maps = [
        {"z": np.ascontiguousarray(zt[:, i * NCOLS:(i + 1) * NCOLS]), "u": u_all}
        for i in range(N_CORES)
    ]
    res = run_bass_kernel_spmd(nc, in_maps, core_ids=list(range(N_CORES)), trace=trace)

    vt = np.concatenate([r["v"] for r in res.results], axis=1)  # [T, B*2] bf16
    out = np.ascontiguousarray(
        vt.astype(np.float32).reshape(T, B, 2).transpose(1, 0, 2)
    )
    return out, res


def kernel(x_seq: np.ndarray) -> np.ndarray:
    out, _ = _run(x_seq, trace=False)
    return out
